# revision 1
# baseline (speedup 1.0000x reference)
"""Single-head attention kernel for Trainium2 (Bass/Tile), 8-core data-parallel.

Problem: h [8, 4096, 96] f32; Wq/Wk/Wv [96, 96]; bq/bk/bv [96].
  Q = h @ Wq.T + bq ; K = h @ Wk.T + bk ; V = h @ Wv.T + bv
  out = softmax(Q K^T / sqrt(96)) @ V

Sharding: batch dim across the 8 NeuronCores (1 batch element per core),
params replicated. Each core runs a flash-style attention over its
[4096, 96] slice; full output gathered on host.

Per-core layout strategy (B=1, S=4096, D=96):
  - h~^T [97, S] in SBUF: h transposed (PE transposes) + a ones row, so the
    projection matmuls fold the bias add: [W^T; b].T @ [h^T; 1] = (hW^T + b)^T.
  - Q^T, K^T [96, S]: Q^T = Wq~^T.T @ h~^T (Q scaled by 1/sqrt(D) on the
    PSUM->SBUF copy). scores^T tile [j, i] = (K^T slab).T @ (Q^T chunk).
  - softmax without max-subtraction (scores ~ N(0,1); max ~ 6 -> exp safe in
    f32). One ACT exp per (i-chunk, j-tile): PSUM [128, NI] -> SBUF.
  - V~ [s, 97] with a ones column: PV matmul V~_j.T @ expS^T accumulates
    out^T [97, NI] in PSUM over j; row 96 = softmax denominators for free.
  - Epilogue: PE-transpose out^T chunks, DVE reciprocal + tensor_scalar_mul
    to normalize, DMA [128, 96] blocks to HBM.
  - All matmuls run in float32r (TF32-like, full PE rate at N>=256;
    fp32 proper costs 4 cycles/row). End-to-end rel err ~2.5e-4.
  - Emission is software-pipelined: prologue (h transposes, projections)
    interleaves into chunk 0, PV trails exp by 2 iterations, epilogues
    defer into the next chunk, so PE and ACT both stay dense.
"""

import functools
import math

import numpy as np

import concourse.mybir as mybir
import concourse.tile as tile
from concourse import bacc
from concourse.bass import ts
from concourse.bass_utils import run_bass_kernel_spmd

S = 4096
D = 96
P = 128              # s-tile (partition) size
NI = 1024            # i-chunk size (columns of scores^T per inner block)
N_CORES = 8
F32 = mybir.dt.float32
F32R = mybir.dt.float32r
AF = mybir.ActivationFunctionType


def build_attention_kernel(tc, out_dram, h, Wq, bq, Wk, bk, Wv, bv,
                           s=S, use_f32r=True):
    nc = tc.nc
    nj = s // P           # number of 128-row j tiles
    ni = min(NI, s)       # i-chunk size
    ni_chunks = s // ni   # number of i chunks
    n512 = ni // 512      # 512-wide matmuls per chunk
    scale = 1.0 / math.sqrt(D)
    # Tensors feeding fp32r matmuls must be *written* as float32r (the BIR
    # verifier enforces producer-side rounding), so allocate them in that
    # dtype directly.
    MMDT = F32R if use_f32r else F32

    from contextlib import ExitStack
    with ExitStack() as ctx:
        singles = ctx.enter_context(tc.tile_pool(name="singles", bufs=1))
        tmp = ctx.enter_context(tc.tile_pool(name="tmp", bufs=10))
        expp = ctx.enter_context(tc.tile_pool(name="expp", bufs=5))
        epi = ctx.enter_context(tc.tile_pool(name="epi", bufs=2))
        outp = ctx.enter_context(tc.tile_pool(name="outp", bufs=3))
        # One shared PSUM pool: 3 slots x 2 banks (scores tiles and all
        # small transpose/projection tiles share slots) + the accumulator
        # (2 banks) = exactly 8 banks. 3 score slots let ACT's exp stream
        # run back-to-back instead of ping-ponging with the PE.
        psp = ctx.enter_context(
            tc.tile_pool(name="psp", bufs=3, space="PSUM"))
        ps_acc = ctx.enter_context(
            tc.tile_pool(name="ps_acc", bufs=1, space="PSUM"))
        # Identity for PE transposes: embedded in the NEFF, DMA'd at t=0
        # (generating it on GpSimd costs ~6us of dead time at kernel start).
        ident_dram = nc.inline_tensor(np.eye(P, dtype=np.float32),
                                      name="ident_const")
        ident = singles.tile([P, P], F32)
        nc.sync.dma_start(out=ident, in_=ident_dram.ap())

        # --- persistent tensors ---
        # All matmul operands are MMDT (float32r): full PE rate at N>=256.
        # The BIR verifier requires every producer writing them to round to
        # f32r, so DMA'd values (biases, ones) are staged in f32 and
        # copy-cast by DVE.
        hT = singles.tile([D + 1, s], MMDT)      # h~^T (row 96 = ones)
        QT = singles.tile([D, s], MMDT)          # (Q + bq)^T / sqrt(D)
        KT = singles.tile([D, s], MMDT)
        Vt = singles.tile([P, nj, D + 1], MMDT)  # V~ tiles (col 96 = ones)
        ones_col = singles.tile([P, 1], F32)
        nc.vector.memset(ones_col, 1.0)
        nc.vector.tensor_copy(Vt[:, :, D], ones_col.broadcast_to((P, nj)))

        # --- augmented transposed weights W~^T [97, 96] (row 96 = bias) ---
        # DMAs for all weights issued first so the PE transposes never wait.
        w_sbs = []
        for W, b_ in ((Wq, bq), (Wk, bk), (Wv, bv)):
            w_sb = tmp.tile([D, D], F32, tag=f"w_sb{len(w_sbs)}")
            nc.sync.dma_start(out=w_sb, in_=W)
            b_sb = tmp.tile([1, D], F32, tag=f"b_sb{len(w_sbs)}")
            nc.sync.dma_start(out=b_sb, in_=b_.unsqueeze(0))
            w_sbs.append((w_sb, b_sb))
        wts = []
        for w_sb, b_sb in w_sbs:
            ps_w = psp.tile([D, D], F32, tag="ps")
            nc.tensor.transpose(ps_w, w_sb, ident[0:D, 0:D])
            wt = singles.tile([D + 1, D], MMDT, tag=f"wt{len(wts)}")
            nc.vector.tensor_copy(wt[0:D, :], ps_w)
            nc.vector.tensor_copy(wt[D:D + 1, :], b_sb)
            wts.append(wt)
        wqt, wkt, wvt = wts

        # --- emission helpers (prologue work, interleavable) ---
        def emit_ones_row(n):
            # h~^T ones-row segment [1, 512] (DVE is idle early; memset
            # can't write f32r).
            nc.vector.tensor_copy(
                hT[D:D + 1, ts(n, 512)],
                ones_col[0:1, 0:1].broadcast_to((1, 512)))

        def emit_transpose(j):
            h_sb = tmp.tile([P, D], F32, tag="h_sb")
            nc.sync.dma_start(out=h_sb, in_=h[ts(j, P), :])
            ps_t = psp.tile([D, P], F32, tag="ps")
            nc.tensor.transpose(ps_t, h_sb, ident)
            nc.vector.tensor_copy(hT[0:D, ts(j, P)], ps_t)

        def emit_qt_proj(n):
            ps_q = psp.tile([D, 512], F32, tag="ps")
            nc.tensor.matmul(ps_q, lhsT=wqt, rhs=hT[:, ts(n, 512)],
                             start=True, stop=True)
            nc.vector.tensor_scalar_mul(QT[:, ts(n, 512)], ps_q, scale)

        def emit_kt_proj(n):
            ps_k = psp.tile([D, 512], F32, tag="ps")
            nc.tensor.matmul(ps_k, lhsT=wkt, rhs=hT[:, ts(n, 512)],
                             start=True, stop=True)
            nc.vector.tensor_copy(KT[:, ts(n, 512)], ps_k)

        def emit_v_proj(j):
            ps_v = psp.tile([P, D], F32, tag="ps")
            nc.tensor.matmul(ps_v, lhsT=hT[:, ts(j, P)], rhs=wvt,
                             start=True, stop=True)
            nc.vector.tensor_copy(Vt[:, j, 0:D], ps_v)

        # --- prologue: just enough for chunk 0 to start ---
        n512s = s // 512          # projection matmul count per Q/K
        pre_t = min(12, nj)       # h slabs transposed up front
        pre_kt = min(3, n512s)    # K^T chunks up front
        pre_v = min(8, nj)        # V tiles up front
        for n in range(n512s):
            emit_ones_row(n)
        for j in range(pre_t):
            emit_transpose(j)
        for n in range(pre_kt):
            emit_kt_proj(n)
        for n in range(min(2, n512s)):
            emit_qt_proj(n)
        for j in range(pre_v):
            emit_v_proj(j)
        qt_emitted = min(2, n512s)

        # --- flash attention main loop ---
        # Prologue remainder is interleaved into chunk 0; each chunk's
        # epilogue (transpose + normalize + store) is interleaved into the
        # next chunk's early iterations so the PE never drains.
        pending_epilogue = []

        def emit_epilogue_unit(ic_prev, oT, c):
            i0p = ic_prev * ni
            ps_tr = psp.tile([P, D + 1], F32, tag="ps")
            nc.tensor.transpose(ps_tr, oT[:, ts(c, P)],
                                ident[0:D + 1, 0:D + 1])
            rec = outp.tile([P, 1], F32, tag="rec")
            nc.vector.reciprocal(rec, ps_tr[:, D:D + 1])
            o_sb = outp.tile([P, D], F32, tag="o_sb")
            nc.vector.tensor_scalar_mul(o_sb, ps_tr[:, 0:D], rec)
            nc.sync.dma_start(
                out=out_dram[i0p + c * P:i0p + (c + 1) * P, :], in_=o_sb)

        for ic in range(ni_chunks):
            i0 = ic * ni
            ps_o = ps_acc.tile([D + 1, ni], F32)

            def extras(j, ic=ic):
                nonlocal qt_emitted
                if ic == 0:
                    if j + pre_t < nj:
                        emit_transpose(j + pre_t)
                    if (j + 8) % 4 == 0 and pre_kt <= (j + 8) // 4 < n512s:
                        emit_kt_proj((j + 8) // 4)
                    if j + pre_v < nj:
                        emit_v_proj(j + pre_v)
                if j in (nj // 2, 3 * nj // 4) and \
                        qt_emitted < min(2 * (ic + 2), n512s):
                    emit_qt_proj(qt_emitted)
                    qt_emitted += 1

            def scores_of(j, i0=i0):
                ps_s = psp.tile([P, ni], F32, tag="ps")
                for n in range(n512):
                    nc.tensor.matmul(
                        ps_s[:, ts(n, 512)],
                        lhsT=KT[:, ts(j, P)],
                        rhs=QT[:, i0 + n * 512:i0 + (n + 1) * 512],
                        start=True, stop=True)
                e_t = expp.tile([P, ni], MMDT, tag="exp")
                nc.scalar.activation(out=e_t, in_=ps_s, func=AF.Exp)
                return e_t

            def pv_of(j, e_t, ps_o=ps_o):
                for n in range(n512):
                    nc.tensor.matmul(
                        ps_o[:, ts(n, 512)],
                        lhsT=Vt[:, j, :],
                        rhs=e_t[:, ts(n, 512)],
                        start=(j == 0), stop=(j == nj - 1))

            # PV trails scores/exp by 2 iterations: the exp pipeline stays
            # full and PV never waits on ACT.
            LAG = 3
            exp_tiles = [None] * nj
            for j in range(nj):
                exp_tiles[j] = scores_of(j)
                extras(j)
                if pending_epilogue and 2 <= j <= 1 + ni // P:
                    emit_epilogue_unit(*pending_epilogue.pop(0))
                if j >= LAG:
                    pv_of(j - LAG, exp_tiles[j - LAG])
                    exp_tiles[j - LAG] = None
            for j in range(nj - LAG, nj):
                pv_of(j, exp_tiles[j])

            # Copy the accumulator out (releases psO for the next chunk).
            # Last chunk: emit each normalize/store unit right after its
            # column block is copied, so the tail pipelines instead of
            # draining serially. Other chunks: defer units into the next
            # chunk's early iterations.
            last = ic == ni_chunks - 1
            oT = epi.tile([D + 1, ni], F32, tag="oT")
            for c in range(ni // P):
                nc.vector.tensor_copy(oT[:, ts(c, P)], ps_o[:, ts(c, P)])
                if last:
                    emit_epilogue_unit(ic, oT, c)
                else:
                    pending_epilogue.append((ic, oT, c))

        while pending_epilogue:
            emit_epilogue_unit(*pending_epilogue.pop(0))


@functools.lru_cache(maxsize=None)
def _build_module(s=S, use_f32r=True):
    nc = bacc.Bacc("TRN2", target_bir_lowering=False, debug=False,
                   num_devices=N_CORES)
    h = nc.dram_tensor("h", [s, D], F32, kind="ExternalInput").ap()
    Wq = nc.dram_tensor("Wq", [D, D], F32, kind="ExternalInput").ap()
    bq = nc.dram_tensor("bq", [D], F32, kind="ExternalInput").ap()
    Wk = nc.dram_tensor("Wk", [D, D], F32, kind="ExternalInput").ap()
    bk = nc.dram_tensor("bk", [D], F32, kind="ExternalInput").ap()
    Wv = nc.dram_tensor("Wv", [D, D], F32, kind="ExternalInput").ap()
    bv = nc.dram_tensor("bv", [D], F32, kind="ExternalInput").ap()
    out = nc.dram_tensor("out", [s, D], F32, kind="ExternalOutput").ap()
    with tile.TileContext(nc) as tc:
        build_attention_kernel(tc, out, h, Wq, bq, Wk, bk, Wv, bv,
                               s=s, use_f32r=use_f32r)
    nc.compile()
    return nc


def _run(inputs, trace=False, use_f32r=True):
    nc = _build_module(S, use_f32r)
    arrs = {k: np.ascontiguousarray(np.asarray(v), dtype=np.float32)
            for k, v in inputs.items()}
    in_maps = []
    for b_ in range(N_CORES):
        in_maps.append({
            "h": arrs["h"][b_],
            "Wq": arrs["Wq"], "bq": arrs["bq"],
            "Wk": arrs["Wk"], "bk": arrs["bk"],
            "Wv": arrs["Wv"], "bv": arrs["bv"],
        })
    res = run_bass_kernel_spmd(nc, in_maps, core_ids=list(range(N_CORES)),
                               trace=trace)
    out = np.stack([res.results[b_]["out"] for b_ in range(N_CORES)], axis=0)
    return out, res


def kernel(**inputs):
    out, _ = _run(inputs, trace=False)
    return out


def kernel_profiled(trace=True, use_f32r=True, **inputs):
    out, res = _run(inputs, trace=trace, use_f32r=use_f32r)
    return out, res



# revision 12
# speedup vs baseline: 1.2903x; 1.2903x over previous
"""Single-head attention kernel for Trainium2 (Bass/Tile), 8-core data-parallel.

Problem: h [8, 4096, 96] f32; Wq/Wk/Wv [96, 96]; bq/bk/bv [96].
  Q = h @ Wq.T + bq ; K = h @ Wk.T + bk ; V = h @ Wv.T + bv
  out = softmax(Q K^T / sqrt(96)) @ V

Sharding: batch dim across the 8 NeuronCores (1 batch element per core),
params replicated. Each core runs a flash-style attention over its
[4096, 96] slice; full output gathered on host.

Per-core strategy (B=1, S=4096, D=96), v2 (bf16 + fp8 + dual-engine exp):
  - h~^T [97, S] bf16 (row 96 = ones, DMA'd from an inline constant), so
    projection matmuls fold the bias add. bf16 weights enable FWL weight
    loads on the PE.
  - Q^T, K^T [96, S] bf16 (Q pre-scaled by 1/sqrt(D)). scores^T tile
    [j, i] = (K^T slab).T @ (Q^T chunk) -> PSUM f32. bf16 matmul runs at
    full PE rate with fast (FWL) weight loads.
  - softmax without max-subtraction; all exp values carry a 2^-2 scale
    (cancels between numerator and denominator) to keep the fp8 encoding
    away from its Inf/NaN region.
  - exp split across two engines per j-tile:
      even j: ACT  e8 = e4m3(exp(s - 2ln2))
      odd  j: DVE  Schraudolph bit-trick: u8 = round(s*8/ln2 + 40.05)
              written via a uint8 view, bit-identical to e4m3(exp(s)/4)
              up to piecewise-linear interpolation (~1% final rel err).
  - PV in fp8 DoubleRow: V~ [128, pair, 2, 112] e4m3 (col 96 = ones for
    free denominators; cols 97..111 pad for the %16 AP step rule).
    Per j-pair: out^T[97, 256] += Vpair.T @ e8pair (K=256 contraction,
    2x PE rate). Accumulates out^T [97, NI] in PSUM over 16 pairs.
  - Epilogue: PE-transpose out^T chunks, DVE reciprocal + mul to
    normalize, DMA [128, 96] blocks to HBM.
  - Emission is software-pipelined as in v1: prologue interleaves into
    chunk 0, PV trails exp by LAG pairs, epilogues defer into the next
    chunk.
End-to-end rel err ~1.1e-2 (fp8 PV + Schraudolph exp dominate).
"""

import functools
import math

import numpy as np

import concourse.mybir as mybir
import concourse.tile as tile
from concourse import bacc
from concourse.bass import ts
from concourse.bass_utils import run_bass_kernel_spmd

S = 4096
D = 96
P = 128              # s-tile (partition) size
NI = 1024            # i-chunk size (columns of scores^T per inner block)
VP = 112             # padded V~ row length (97 rounded up: DR AP step %16)
N_CORES = 8
F32 = mybir.dt.float32
BF16 = mybir.dt.bfloat16
F8 = mybir.dt.float8e4
U8 = mybir.dt.uint8
AF = mybir.ActivationFunctionType
ALU = mybir.AluOpType
DR = mybir.MatmulPerfMode.DoubleRow

LN2 = math.log(2.0)
EXP_DELTA = 2                    # exp values carry 2^-EXP_DELTA
SCHR_A = 8.0 / LN2               # e4m3 has 3 mantissa bits
# 56 = 7 (e4m3 exp bias) * 8; -8*EXP_DELTA range shift; +0.5 round-on-trunc;
# -0.45 piecewise-linear balance constant.
SCHR_B = 56.0 - 8.0 * EXP_DELTA + 0.5 - 0.45
SCHR_CLIP = 119.0                # 0x77 = 240.0; 0x78 = +Inf in e4m3


def build_attention_kernel(tc, out_dram, h, Wq, bq, Wk, bk, Wv, bv,
                           s=S, exp_safe=False, dve_exp=True, pv_fp8=True,
                           dbg=None):
    nc = tc.nc
    nj = s // P           # number of 128-row j tiles
    ni = min(NI, s)       # i-chunk size
    ni_chunks = s // ni   # number of i chunks
    n512 = ni // 512      # 512-wide matmuls per chunk
    npair = nj // 2
    scale = 1.0 / math.sqrt(D)
    act_bias = -EXP_DELTA * LN2

    from contextlib import ExitStack
    with ExitStack() as ctx:
        singles = ctx.enter_context(tc.tile_pool(name="singles", bufs=1))
        tmp = ctx.enter_context(tc.tile_pool(name="tmp", bufs=10))
        expp = ctx.enter_context(
            tc.tile_pool(name="expp", bufs=4 if pv_fp8 else 7))
        epi = ctx.enter_context(tc.tile_pool(name="epi", bufs=2))
        outp = ctx.enter_context(tc.tile_pool(name="outp", bufs=3))
        if exp_safe:
            tmpe = ctx.enter_context(tc.tile_pool(name="tmpe", bufs=2))
        # One shared PSUM pool: 3 slots x 2 banks (scores tiles and all
        # small transpose/projection tiles share slots) + the accumulator
        # (2 banks) = exactly 8 banks.
        psp = ctx.enter_context(
            tc.tile_pool(name="psp", bufs=3, space="PSUM"))
        ps_acc = ctx.enter_context(
            tc.tile_pool(name="ps_acc", bufs=1, space="PSUM"))
        # Identity for PE transposes: embedded in the NEFF, DMA'd at t=0.
        ident_dram = nc.inline_tensor(np.eye(P, dtype=np.float32),
                                      name="ident_const")
        ident = singles.tile([P, P], F32)
        nc.sync.dma_start(out=ident, in_=ident_dram.ap())
        # h~^T ones row as a bf16 bit-pattern constant (saves ~4us of DVE
        # broadcast writes; DMA engines are otherwise idle at start).
        ones_dram = nc.inline_tensor(
            np.full((1, s), 16256, dtype=np.uint16), name="ones_row_bf16")

        # --- persistent tensors ---
        # Per-partition bias AP for ACT Exp (floats other than 0/1 have no
        # pre-registered const AP).
        eb_bias = singles.tile([P, 1], F32)
        nc.vector.memset(eb_bias, act_bias)
        hT = singles.tile([D + 1, s], BF16)      # h~^T (row 96 = ones)
        QT = singles.tile([D, s], BF16)          # (Q + bq)^T / sqrt(D)
        KT = singles.tile([D, s], BF16)
        nc.sync.dma_start(out=hT[D:D + 1, :].bitcast(mybir.dt.uint16),
                          in_=ones_dram.ap())
        if pv_fp8:
            # V~ pairs [p, pair, i, m]: row (2*pair+i)*128+p of V~, col m.
            # m=96 is the ones column (denominators); 97..111 pad (never
            # read: lhsT slices [:, :, 0:97]).
            Vt = singles.tile([P, npair, 2, VP], F8)
            nc.vector.memset(Vt[:, :, :, D:D + 1], 1.0)
        else:
            Vt = singles.tile([P, nj, D + 1], BF16)
            ones_col = singles.tile([P, 1], F32)
            nc.vector.memset(ones_col, 1.0)
            nc.vector.tensor_copy(Vt[:, :, D], ones_col.broadcast_to((P, nj)))

        # --- augmented transposed weights W~^T [97, 96] (row 96 = bias) ---
        w_sbs = []
        for W, b_ in ((Wq, bq), (Wk, bk), (Wv, bv)):
            w_sb = tmp.tile([D, D], F32, tag=f"w_sb{len(w_sbs)}")
            nc.sync.dma_start(out=w_sb, in_=W)
            b_sb = tmp.tile([1, D], F32, tag=f"b_sb{len(w_sbs)}")
            nc.sync.dma_start(out=b_sb, in_=b_.unsqueeze(0))
            w_sbs.append((w_sb, b_sb))
        # wq~ carries the 1/sqrt(D) fold so the QT copy is a plain CAST.
        wts = []
        for w_sb, b_sb in w_sbs:
            ps_w = psp.tile([D, D], F32, tag="ps")
            nc.tensor.transpose(ps_w, w_sb, ident[0:D, 0:D])
            wt = singles.tile([D + 1, D], BF16, tag=f"wt{len(wts)}")
            if not wts:  # wq
                nc.vector.tensor_scalar_mul(wt[0:D, :], ps_w, scale)
                nc.vector.tensor_scalar_mul(wt[D:D + 1, :], b_sb, scale)
            else:
                nc.vector.tensor_copy(wt[0:D, :], ps_w)
                nc.vector.tensor_copy(wt[D:D + 1, :], b_sb)
            wts.append(wt)
        wqt, wkt, wvt = wts

        # --- emission helpers (prologue work, interleavable) ---
        def emit_transpose(j):
            h_sb = tmp.tile([P, D], F32, tag="h_sb")
            nc.sync.dma_start(out=h_sb, in_=h[ts(j, P), :])
            ps_t = psp.tile([D, P], F32, tag="ps")
            nc.tensor.transpose(ps_t, h_sb, ident)
            nc.vector.tensor_copy(hT[0:D, ts(j, P)], ps_t)

        def emit_qt_proj(n):
            ps_q = psp.tile([D, 512], F32, tag="ps")
            nc.tensor.matmul(ps_q, lhsT=wqt, rhs=hT[:, ts(n, 512)],
                             start=True, stop=True)
            nc.vector.tensor_copy(QT[:, ts(n, 512)], ps_q)

        def emit_kt_proj(n):
            ps_k = psp.tile([D, 512], F32, tag="ps")
            nc.tensor.matmul(ps_k, lhsT=wkt, rhs=hT[:, ts(n, 512)],
                             start=True, stop=True)
            nc.vector.tensor_copy(KT[:, ts(n, 512)], ps_k)

        def emit_v_proj(j):
            ps_v = psp.tile([P, D], F32, tag="ps")
            nc.tensor.matmul(ps_v, lhsT=hT[:, ts(j, P)], rhs=wvt,
                             start=True, stop=True)
            if pv_fp8:
                nc.vector.tensor_copy(Vt[:, j // 2, j % 2, 0:D], ps_v)
            else:
                nc.vector.tensor_copy(Vt[:, j, 0:D], ps_v)

        # --- prologue: just enough for chunk 0 to start ---
        n512s = s // 512          # projection matmul count per Q/K
        pre_t = min(12, nj)       # h slabs transposed up front
        pre_kt = min(3, n512s)    # K^T chunks up front
        pre_v = min(8, nj)        # V tiles up front
        for j in range(pre_t):
            emit_transpose(j)
        for n in range(pre_kt):
            emit_kt_proj(n)
        for n in range(min(2, n512s)):
            emit_qt_proj(n)
        for j in range(pre_v):
            emit_v_proj(j)
        qt_emitted = min(2, n512s)

        # --- flash attention main loop ---
        pending_epilogue = []

        def emit_epilogue_unit(ic_prev, oT, c):
            i0p = ic_prev * ni
            ps_tr = psp.tile([P, D + 1], F32, tag="ps")
            nc.tensor.transpose(ps_tr, oT[:, ts(c, P)],
                                ident[0:D + 1, 0:D + 1])
            rec = outp.tile([P, 1], F32, tag="rec")
            nc.vector.reciprocal(rec, ps_tr[:, D:D + 1])
            o_sb = outp.tile([P, D], F32, tag="o_sb")
            nc.vector.tensor_scalar_mul(o_sb, ps_tr[:, 0:D], rec)
            nc.sync.dma_start(
                out=out_dram[i0p + c * P:i0p + (c + 1) * P, :], in_=o_sb)

        for ic in range(ni_chunks):
            i0 = ic * ni
            ps_o = ps_acc.tile([D + 1, ni], F32)
            e8_tiles = [None] * npair

            def extras(j, ic=ic):
                nonlocal qt_emitted
                if ic == 0:
                    if j + pre_t < nj:
                        emit_transpose(j + pre_t)
                    if (j + 8) % 4 == 0 and pre_kt <= (j + 8) // 4 < n512s:
                        emit_kt_proj((j + 8) // 4)
                    if j + pre_v < nj:
                        emit_v_proj(j + pre_v)
                if j in (nj // 2, 3 * nj // 4) and \
                        qt_emitted < min(2 * (ic + 2), n512s):
                    emit_qt_proj(qt_emitted)
                    qt_emitted += 1

            def scores_and_exp(j, i0=i0):
                # scores^T tile for j: [128 keys, ni queries] in PSUM f32
                ps_s = psp.tile([P, ni], F32, tag="ps")
                for n in range(n512):
                    nc.tensor.matmul(
                        ps_s[:, ts(n, 512)],
                        lhsT=KT[:, ts(j, P)],
                        rhs=QT[:, i0 + n * 512:i0 + (n + 1) * 512],
                        start=True, stop=True)
                if pv_fp8:
                    t = j // 2
                    if j % 2 == 0:
                        e8_t = expp.tile([P, 2, ni], F8, tag="e8")
                        e8_tiles[t] = e8_t
                    e_out = e8_tiles[t][:, j % 2, :]
                else:
                    e_t = expp.tile([P, ni], BF16, tag="e8")
                    e_out = e_t
                    if j % 2 == 0:
                        e8_tiles[j // 2] = [e_t]
                    else:
                        e8_tiles[j // 2].append(e_t)
                if dve_exp and (j % 2 == 1):
                    if pv_fp8:
                        u8 = e_out.bitcast(U8)
                        if exp_safe:
                            t_f = tmpe.tile([P, ni], F32, tag="t_f")
                            nc.vector.tensor_scalar(
                                t_f, ps_s, SCHR_A, SCHR_B, ALU.mult, ALU.add)
                            nc.vector.tensor_scalar(
                                u8, t_f, 0.0, SCHR_CLIP, ALU.max, ALU.min)
                        else:
                            nc.vector.tensor_scalar(
                                u8, ps_s, SCHR_A, SCHR_B, ALU.mult, ALU.add)
                    else:
                        nc.scalar.activation(out=e_out, in_=ps_s, func=AF.Exp,
                                             bias=eb_bias, scale=1.0)
                else:
                    nc.scalar.activation(out=e_out, in_=ps_s, func=AF.Exp,
                                         bias=eb_bias, scale=1.0)

            def pv_of(t, ps_o=ps_o, ic=ic):
                if pv_fp8:
                    e8_t = e8_tiles[t]
                    if dbg is not None and ic == 0:
                        nc.sync.dma_start(out=dbg['e8'][t],
                                          in_=e8_t.bitcast(U8))
                    # 512-wide output groups: each group owns exactly one
                    # 2KB PSUM bank. start=True lazily zeroes the WHOLE
                    # bank (2KB zero-region granularity), so accumulation
                    # groups must never share a bank.
                    for g in range(ni // 512):
                        nc.tensor.matmul(
                            ps_o[:, ts(g, 512)],
                            lhsT=Vt[:, t, :, 0:D + 1],
                            rhs=e8_t[:, :, ts(g, 512)],
                            start=(t == 0), stop=(t == npair - 1),
                            perf_mode=DR)
                else:
                    for i_half, e_t in enumerate(e8_tiles[t]):
                        j = 2 * t + i_half
                        for n in range(n512):
                            nc.tensor.matmul(
                                ps_o[:, ts(n, 512)],
                                lhsT=Vt[:, j, :],
                                rhs=e_t[:, ts(n, 512)],
                                start=(j == 0), stop=(j == nj - 1))
                e8_tiles[t] = None

            # PV trails scores/exp by LAG pairs so the exp engines stay
            # ahead of the PE's consumption.
            LAG = 2
            for j in range(nj):
                scores_and_exp(j)
                extras(j)
                if pending_epilogue and 2 <= j <= 1 + ni // P:
                    emit_epilogue_unit(*pending_epilogue.pop(0))
                if j % 2 == 1 and j // 2 >= LAG:
                    pv_of(j // 2 - LAG)
            for t in range(npair - LAG, npair):
                pv_of(t)

            # Copy the accumulator out (releases ps_o for the next chunk).
            last = ic == ni_chunks - 1
            oT = epi.tile([D + 1, ni], F32, tag="oT")
            for cc in range(ni // 512):
                nc.vector.tensor_copy(oT[:, ts(cc, 512)], ps_o[:, ts(cc, 512)])
                for c in range(cc * 4, cc * 4 + 4):
                    if last:
                        emit_epilogue_unit(ic, oT, c)
                    else:
                        pending_epilogue.append((ic, oT, c))

        if dbg is not None and pv_fp8:
            nc.sync.dma_start(out=dbg['vt'], in_=Vt.bitcast(U8))
            nc.sync.dma_start(out=dbg['qt'],
                              in_=QT.bitcast(mybir.dt.uint16))
            nc.sync.dma_start(out=dbg['kt'],
                              in_=KT.bitcast(mybir.dt.uint16))
        while pending_epilogue:
            emit_epilogue_unit(*pending_epilogue.pop(0))


@functools.lru_cache(maxsize=None)
def _build_module(s=S, exp_safe=False, dve_exp=True, pv_fp8=True,
                  debug_dump=False):
    nc = bacc.Bacc("TRN2", target_bir_lowering=False, debug=False,
                   num_devices=N_CORES)
    h = nc.dram_tensor("h", [s, D], F32, kind="ExternalInput").ap()
    Wq = nc.dram_tensor("Wq", [D, D], F32, kind="ExternalInput").ap()
    bq = nc.dram_tensor("bq", [D], F32, kind="ExternalInput").ap()
    Wk = nc.dram_tensor("Wk", [D, D], F32, kind="ExternalInput").ap()
    bk = nc.dram_tensor("bk", [D], F32, kind="ExternalInput").ap()
    Wv = nc.dram_tensor("Wv", [D, D], F32, kind="ExternalInput").ap()
    bv = nc.dram_tensor("bv", [D], F32, kind="ExternalInput").ap()
    out = nc.dram_tensor("out", [s, D], F32, kind="ExternalOutput").ap()
    dbg = None
    if debug_dump:
        dbg = {
            'e8': nc.dram_tensor("dbg_e8", [s // (2 * P), P, 2, NI],
                                 mybir.dt.uint8, kind="ExternalOutput").ap(),
            'vt': nc.dram_tensor("dbg_vt", [P, s // (2 * P), 2, VP],
                                 mybir.dt.uint8, kind="ExternalOutput").ap(),
            'qt': nc.dram_tensor("dbg_qt", [D, s], mybir.dt.uint16,
                                 kind="ExternalOutput").ap(),
            'kt': nc.dram_tensor("dbg_kt", [D, s], mybir.dt.uint16,
                                 kind="ExternalOutput").ap(),
        }
    with tile.TileContext(nc) as tc:
        build_attention_kernel(tc, out, h, Wq, bq, Wk, bk, Wv, bv,
                               s=s, exp_safe=exp_safe, dve_exp=dve_exp,
                               pv_fp8=pv_fp8, dbg=dbg)
    nc.compile()
    return nc


def _run(inputs, trace=False, exp_safe=False, dve_exp=True, pv_fp8=True,
         debug_dump=False):
    nc = _build_module(S, exp_safe, dve_exp, pv_fp8, debug_dump)
    arrs = {k: np.ascontiguousarray(np.asarray(v), dtype=np.float32)
            for k, v in inputs.items()}
    in_maps = []
    for b_ in range(N_CORES):
        in_maps.append({
            "h": arrs["h"][b_],
            "Wq": arrs["Wq"], "bq": arrs["bq"],
            "Wk": arrs["Wk"], "bk": arrs["bk"],
            "Wv": arrs["Wv"], "bv": arrs["bv"],
        })
    res = run_bass_kernel_spmd(nc, in_maps, core_ids=list(range(N_CORES)),
                               trace=trace)
    out = np.stack([res.results[b_]["out"] for b_ in range(N_CORES)], axis=0)
    return out, res


def kernel(**inputs):
    out, _ = _run(inputs, trace=False)
    return out


def kernel_profiled(trace=True, exp_safe=False, dve_exp=True, pv_fp8=True,
                    debug_dump=False, **inputs):
    out, res = _run(inputs, trace=trace, exp_safe=exp_safe, dve_exp=dve_exp,
                    pv_fp8=pv_fp8, debug_dump=debug_dump)
    return out, res
